# revision 1
# baseline (speedup 1.0000x reference)
"""GAT (5-layer, 41 heads, max-aggr) on 8 trn2 NeuronCores.

Strategy (dst-sharded graph parallel):
  - nodes are sharded contiguously across the 8 cores (12500 each, padded
    to 12544 = 98*128); within a core, nodes are sorted by in-degree so
    that ELL tiles of 128 nodes have near-uniform segment length.
  - per layer: each core computes z = act @ W for its own nodes (TensorE),
    writes its shard of the feature table to DRAM and AllGathers the full
    table; per 128-node tile the incoming-edge source rows are fetched with
    one indirect DMA (row gather) in [128, D_t, 41] ELL layout; the
    segment softmax + max-aggregation reduce along the free axis (VectorE).
  - leaky-relu/exp run on ScalarE; per-tile scalars are batched into
    layer-wide [128, 98*41] passes to amortize instruction overhead.
"""
import sys
for _p in ("/opt/trn_rl_repo",):
    if _p not in sys.path:
        sys.path.insert(0, _p)

import numpy as np
from contextlib import ExitStack

from concourse import bass, mybir, tile, bacc, bass_utils
from concourse.masks import make_identity

F32 = mybir.dt.float32
I32 = mybir.dt.int32
ALU = mybir.AluOpType
AF = mybir.ActivationFunctionType
AX = mybir.AxisListType

P = 128
NEG_SLOPE = 0.2


class Cfg:
    def __init__(self, n_cores=8, nodes_real=12500, n_tiles=98, f_in=602,
                 f_in_pad=640, h=41, L=5, mchunk=512, use_act_lrelu=True,
                 reps=1, ablate="", slot_budget=1, max_group=16, gbarrier=False):
        self.n_cores = n_cores
        self.nodes_real = nodes_real          # real nodes per core
        self.n_tiles = n_tiles                # 128-node tiles per core
        self.nodes_pad = n_tiles * P          # padded nodes per core
        self.f_in = f_in
        self.f_in_pad = f_in_pad              # multiple of 128
        self.kt = f_in_pad // P               # k-tiles for layer 0
        self.h = h                            # heads (= feature width)
        self.L = L
        self.mchunk = mchunk                  # matmul N-chunk (<=512)
        self.vg = self.nodes_pad * n_cores    # global (padded) node count
        self.use_act_lrelu = use_act_lrelu
        self.reps = reps
        self.ablate = ablate
        self.slot_budget = slot_budget
        self.max_group = max_group
        self.gbarrier = gbarrier


# ----------------------------------------------------------------- builder --
def build_nc(cfg, Dts, groups):
    """Build the SPMD Bass program (grouped gathers, per-tile compute)."""
    nt, h, L = cfg.n_tiles, cfg.h, cfg.L
    slot_cols = int(sum(Dts))
    d_max = int(max(Dts))
    gslot_max = int(max(gn * Dg for (_t0, gn, Dg) in groups))

    nc = bacc.Bacc("TRN2", target_bir_lowering=False, debug=False,
                   num_devices=cfg.n_cores)

    xT = nc.dram_tensor("xT", [cfg.f_in_pad, cfg.nodes_pad], F32, kind="ExternalInput")
    idxs = nc.dram_tensor("idxs", [P, slot_cols], I32, kind="ExternalInput")
    npad = nc.dram_tensor("npad", [P, nt], F32, kind="ExternalInput")
    w0 = nc.dram_tensor("w0", [cfg.f_in_pad, h], F32, kind="ExternalInput")
    wrest = nc.dram_tensor("wrest", [max(L - 1, 1) * h, h], F32, kind="ExternalInput")
    asrep = nc.dram_tensor("asrep", [L * P, h], F32, kind="ExternalInput")
    adrep = nc.dram_tensor("adrep", [L * P, h], F32, kind="ExternalInput")
    brep = nc.dram_tensor("brep", [L * P, h], F32, kind="ExternalInput")
    out_d = nc.dram_tensor("out", [cfg.nodes_pad, h], F32, kind="ExternalOutput")

    groups_rg = [list(range(cfg.n_cores))]

    with tile.TileContext(nc) as tc, ExitStack() as ctx:
        const = ctx.enter_context(tc.tile_pool(name="const", bufs=1))
        actp = ctx.enter_context(tc.tile_pool(name="actp", bufs=1))
        zp = ctx.enter_context(tc.tile_pool(name="zp", bufs=1))
        widep = ctx.enter_context(tc.tile_pool(name="widep", bufs=1))
        rhsp = ctx.enter_context(tc.tile_pool(name="rhsp", bufs=3))
        ztp = ctx.enter_context(tc.tile_pool(name="ztp", bufs=2))
        gp = ctx.enter_context(tc.tile_pool(name="gp", bufs=2))
        ep = ctx.enter_context(tc.tile_pool(name="ep", bufs=2))
        tp = ctx.enter_context(tc.tile_pool(name="tp", bufs=2))
        smp = ctx.enter_context(tc.tile_pool(name="smp", bufs=3))
        psmm = ctx.enter_context(tc.tile_pool(name="psmm", bufs=2, space="PSUM"))
        pstr = ctx.enter_context(tc.tile_pool(name="pstr", bufs=3, space="PSUM"))
        pstr2 = ctx.enter_context(tc.tile_pool(name="pstr2", bufs=2, space="PSUM"))
        dram = ctx.enter_context(tc.tile_pool(name="dram", bufs=2, space="DRAM"))

        # constants
        ident = const.tile([P, P], F32)
        make_identity(nc, ident[:])
        idx_sb = const.tile([P, slot_cols], I32)
        nc.sync.dma_start(out=idx_sb[:], in_=idxs[:])
        npad_sb = const.tile([P, nt], F32)
        nc.sync.dma_start(out=npad_sb[:], in_=npad[:])
        as_sb = const.tile([P, L * h], F32)
        nc.sync.dma_start(out=as_sb[:].rearrange("p (l h) -> p l h", l=L), in_=asrep[:].rearrange("(l p) h -> p l h", p=P))
        ad_sb = const.tile([P, L * h], F32)
        nc.sync.dma_start(out=ad_sb[:].rearrange("p (l h) -> p l h", l=L), in_=adrep[:].rearrange("(l p) h -> p l h", p=P))
        b_sb = const.tile([P, L * h], F32)
        nc.sync.dma_start(out=b_sb[:].rearrange("p (l h) -> p l h", l=L), in_=brep[:].rearrange("(l p) h -> p l h", p=P))
        w0_sb = const.tile([P, cfg.kt * h], F32)
        nc.sync.dma_start(out=w0_sb[:].rearrange("p (k h) -> p k h", k=cfg.kt), in_=w0[:].rearrange("(k p) h -> p k h", p=P))
        wr_sb = const.tile([h, max(L - 1, 1) * h], F32)
        nc.sync.dma_start(out=wr_sb[:].rearrange("p (l h) -> p l h", l=max(L - 1, 1)), in_=wrest[:].rearrange("(l p) h -> p l h", p=h))

        # slot-column offsets per tile
        offs = np.concatenate([[0], np.cumsum(Dts)]).astype(int)

        # m-chunk list for the node dimension
        mlist = []
        m0 = 0
        while m0 < cfg.nodes_pad:
            mw = min(cfg.mchunk, cfg.nodes_pad - m0)
            mlist.append((m0, mw))
            m0 += mw

        def stage_matmul(l, actT):
            """z = act @ W_l -> z_sb [P, nt*h] (node-major) + AllGather table."""
            z_sb = zp.tile([P, nt * h], F32, tag="z_sb")
            for (m0, mw) in mlist:
                ps = psmm.tile([h, cfg.mchunk], F32, tag="mm")
                if l == 0:
                    for k in range(cfg.kt):
                        rhs = rhsp.tile([P, cfg.mchunk], F32, tag="rhs")
                        nc.sync.dma_start(out=rhs[:, :mw],
                                          in_=xT[k * P:(k + 1) * P, m0:m0 + mw])
                        nc.tensor.matmul(ps[:, :mw], lhsT=w0_sb[:, k * h:(k + 1) * h],
                                         rhs=rhs[:, :mw], start=(k == 0),
                                         stop=(k == cfg.kt - 1))
                else:
                    nc.tensor.matmul(ps[:, :mw], lhsT=wr_sb[:, (l - 1) * h:l * h],
                                     rhs=actT[:, m0:m0 + mw], start=True, stop=True)
                zt = ztp.tile([h, cfg.mchunk], F32, tag="zt")
                nc.scalar.copy(out=zt[:, :mw], in_=ps[:, :mw])
                njt = mw // P
                pt = pstr.tile([P, 4 * h], F32, tag="ztr")
                for j in range(njt):
                    nc.tensor.transpose(out=pt[:, j * h:(j + 1) * h],
                                        in_=zt[:, j * P:(j + 1) * P],
                                        identity=ident[:h, :h])
                t_idx = m0 // P
                nc.scalar.copy(out=z_sb[:, t_idx * h:(t_idx + njt) * h],
                               in_=pt[:, :njt * h])
            bounce = dram.tile([cfg.nodes_pad, h], F32, tag="bounce")
            table = dram.tile([cfg.vg, h], F32, tag="table",
                              addr_space="Shared" if cfg.n_cores > 4 else "Local")
            nc.sync.dma_start(
                out=bounce[:].rearrange("(t p) h -> p t h", p=P),
                in_=z_sb[:].rearrange("p (t h) -> p t h", t=nt))
            nc.gpsimd.collective_compute(
                "AllGather", ALU.bypass, replica_groups=groups_rg,
                ins=[bounce.opt()], outs=[table.opt()])
            return z_sb, table

        def stage_edges(l, z_sb, table):
            """edge softmax + max aggregation; returns out_all [P, nt*h]."""
            a_sl = as_sb[:, l * h:(l + 1) * h]
            a_dl = ad_sb[:, l * h:(l + 1) * h]
            # ad_all = z * a_d (batched)
            ad_all = widep.tile([P, nt * h], F32, tag="ad_all")
            nc.vector.tensor_tensor(
                out=ad_all[:].rearrange("p (t h) -> p t h", t=nt),
                in0=z_sb[:].rearrange("p (t h) -> p t h", t=nt),
                in1=a_dl.unsqueeze(1).broadcast_to([P, nt, h]), op=ALU.mult)
            s_all = widep.tile([P, nt * h], F32, tag="s_all")
            m_all = widep.tile([P, nt * h], F32, tag="m_all")
            ex0_all = widep.tile([P, nt * h], F32, tag="ex0_all")
            for (t0g, gn, Dg) in groups:
              SD = gn * Dg
              gg_t = gp.tile([P, gslot_max * h], F32, tag="g")
              nc.gpsimd.indirect_dma_start(
                  out=gg_t[:, :SD * h], out_offset=None, in_=table[:],
                  in_offset=bass.IndirectOffsetOnAxis(
                      ap=idx_sb[:, offs[t0g]:offs[t0g] + SD], axis=0))
              if cfg.gbarrier and gn > 1:
                  gc_t = ep.tile([P, gslot_max * h], F32, tag="gc")
                  nc.vector.tensor_copy(out=gc_t[:, :SD * h],
                                        in_=gg_t[:, :SD * h])
                  gg_t = gc_t
              for t in range(t0g, t0g + gn):
                D = int(Dts[t])
                loc = (offs[t] - offs[t0g]) * h
                g_ap = gg_t[:, loc:loc + D * h]
                g3 = g_ap.rearrange("p (d h) -> p d h", d=D)
                e_t = ep.tile([P, d_max * h], F32, tag="e")
                e_ap = e_t[:, :D * h]
                e3 = e_ap.rearrange("p (d h) -> p d h", d=D)
                nc.vector.tensor_tensor(out=e3, in0=g3,
                                        in1=a_sl.unsqueeze(1).broadcast_to([P, D, h]),
                                        op=ALU.mult)
                nc.vector.tensor_tensor(
                    out=e3, in0=e3,
                    in1=ad_all[:, t * h:(t + 1) * h].unsqueeze(1).broadcast_to([P, D, h]),
                    op=ALU.add)
                if cfg.use_act_lrelu:
                    nc.scalar.activation(out=e_ap, in_=e_ap, func=AF.Lrelu,
                                         alpha=NEG_SLOPE)
                else:
                    tmp_t = tp.tile([P, d_max * h], F32, tag="tmp")
                    nc.vector.tensor_scalar_mul(out=tmp_t[:, :D * h], in0=e_ap,
                                                scalar1=NEG_SLOPE)
                    nc.vector.tensor_tensor(out=e_ap, in0=e_ap, in1=tmp_t[:, :D * h],
                                            op=ALU.max)
                nc.scalar.activation(out=e_ap, in_=e_ap, func=AF.Exp)
                # ex0 (slot 0) for the padding correction
                nc.scalar.copy(out=ex0_all[:, t * h:(t + 1) * h], in_=e_t[:, :h])
                if cfg.ablate == "nored":
                    nc.scalar.copy(out=s_all[:, t * h:(t + 1) * h], in_=e_t[:, :h])
                    nc.scalar.copy(out=m_all[:, t * h:(t + 1) * h], in_=g_t[:, :h])
                else:
                    # denom = sum_d ex ; msg max = max_d (ex * g)
                    nc.vector.tensor_reduce(out=s_all[:, t * h:(t + 1) * h],
                                            in_=e3.transpose([0, 2, 1]), axis=AX.X,
                                            op=ALU.add)
                    nc.vector.tensor_tensor(out=g_ap, in0=e_ap, in1=g_ap, op=ALU.mult)
                    nc.vector.tensor_reduce(out=m_all[:, t * h:(t + 1) * h],
                                            in_=g3.transpose([0, 2, 1]), axis=AX.X,
                                            op=ALU.max)
            # batched tail: denom -= npad*ex0 ; out = m/denom + b ; act
            w3 = lambda ap: ap.rearrange("p (t h) -> p t h", t=nt)
            npb = npad_sb[:].unsqueeze(2).broadcast_to([P, nt, h])
            nc.vector.tensor_tensor(out=w3(ex0_all[:]), in0=w3(ex0_all[:]), in1=npb,
                                    op=ALU.mult)
            nc.vector.tensor_tensor(out=s_all[:], in0=s_all[:], in1=ex0_all[:],
                                    op=ALU.subtract)
            nc.vector.reciprocal_approx_fast(out=s_all[:], in_=s_all[:])
            out_all = widep.tile([P, nt * h], F32, tag="out_all")
            nc.vector.tensor_tensor(out=out_all[:], in0=m_all[:], in1=s_all[:],
                                    op=ALU.mult)
            b_l = b_sb[:, l * h:(l + 1) * h]
            nc.vector.tensor_tensor(out=w3(out_all[:]), in0=w3(out_all[:]),
                                    in1=b_l.unsqueeze(1).broadcast_to([P, nt, h]),
                                    op=ALU.add)
            if l < L - 1:
                nc.scalar.activation(out=out_all[:], in_=out_all[:], func=AF.Relu)
            return out_all

        def stage_actT(out_all):
            actT = actp.tile([h, cfg.nodes_pad], F32, tag="actT")
            for t0 in range(0, nt, 4):
                gn = min(4, nt - t0)
                pt = pstr2.tile([h, 4 * P], F32, tag="atr")
                for j in range(gn):
                    nc.tensor.transpose(
                        out=pt[:, j * P:(j + 1) * P],
                        in_=out_all[:, (t0 + j) * h:(t0 + j + 1) * h],
                        identity=ident[:])
                nc.scalar.copy(out=actT[:, t0 * P:(t0 + gn) * P],
                               in_=pt[:, :gn * P])
            return actT

        def stage_logsoftmax(out_all):
            w3 = lambda ap: ap.rearrange("p (t h) -> p t h", t=nt)
            mx = smp.tile([P, nt], F32, tag="mx")
            nc.vector.tensor_reduce(out=mx[:], in_=w3(out_all[:]), axis=AX.X,
                                    op=ALU.max)
            mxb = mx[:].unsqueeze(2).broadcast_to([P, nt, h])
            nc.vector.tensor_tensor(out=w3(out_all[:]), in0=w3(out_all[:]), in1=mxb,
                                    op=ALU.subtract)
            exl = widep.tile([P, nt * h], F32, tag="ad_all")
            nc.scalar.activation(out=exl[:], in_=out_all[:], func=AF.Exp)
            sl = smp.tile([P, nt], F32, tag="sl")
            nc.vector.tensor_reduce(out=sl[:], in_=w3(exl[:]), axis=AX.X, op=ALU.add)
            nc.scalar.activation(out=sl[:], in_=sl[:], func=AF.Ln)
            slb = sl[:].unsqueeze(2).broadcast_to([P, nt, h])
            nc.vector.tensor_tensor(out=w3(out_all[:]), in0=w3(out_all[:]), in1=slb,
                                    op=ALU.subtract)
            nc.sync.dma_start(out=out_d[:].rearrange("(t p) h -> p t h", p=P),
                              in_=w3(out_all[:]))

        for _rep in range(cfg.reps):
            actT = None
            for l in range(L):
                z_sb, table = stage_matmul(l, actT)
                if cfg.ablate == "noedge":
                    out_all = z_sb
                else:
                    out_all = stage_edges(l, z_sb, table)
                if l < L - 1:
                    actT = stage_actT(out_all)
                else:
                    stage_logsoftmax(out_all)

    nc.compile()
    return nc


# ------------------------------------------------------------ preprocessing --
def preprocess(edge_index, cfg):
    """Shard + degree-sort + ELL-pack the graph. Returns per-core arrays."""
    n_real = cfg.nodes_real * cfg.n_cores
    src = np.concatenate([edge_index[0], np.arange(n_real, dtype=np.int64)])
    dst = np.concatenate([edge_index[1], np.arange(n_real, dtype=np.int64)])
    deg = np.bincount(dst, minlength=n_real)

    # per-core degree sort -> orders, gid mapping
    orders = []
    gid_of_node = np.empty(n_real, dtype=np.int64)
    for c in range(cfg.n_cores):
        lo = c * cfg.nodes_real
        d = deg[lo:lo + cfg.nodes_real]
        order = np.argsort(-d, kind="stable")          # sorted_pos -> local node
        orders.append(order)
        gid_of_node[lo + order] = c * cfg.nodes_pad + np.arange(cfg.nodes_real)

    # per-tile ELL width, unified across cores
    Dts = np.zeros(cfg.n_tiles, dtype=np.int64)
    deg_sorted = []
    for c in range(cfg.n_cores):
        lo = c * cfg.nodes_real
        ds = deg[lo:lo + cfg.nodes_real][orders[c]]
        ds = np.concatenate([ds, np.zeros(cfg.nodes_pad - cfg.nodes_real, np.int64)])
        deg_sorted.append(ds)
        Dts = np.maximum(Dts, ds.reshape(cfg.n_tiles, P).max(1))
    Dts = np.maximum(Dts, 1)

    groups = []
    t = 0
    while t < cfg.n_tiles:
        Dg = int(Dts[t])
        n = 1
        while (t + n < cfg.n_tiles and n < cfg.max_group
               and (n + 1) * Dg <= cfg.slot_budget):
            n += 1
        groups.append((t, n, Dg))
        Dts[t:t + n] = Dg
        t += n

    offs = np.concatenate([[0], np.cumsum(Dts)]).astype(int)
    slot_cols = int(offs[-1])

    owner = dst // cfg.nodes_real
    src_gid = gid_of_node[src]
    dst_gid = gid_of_node[dst]

    idxs_all, npad_all = [], []
    for c in range(cfg.n_cores):
        mask = owner == c
        sg = src_gid[mask]
        dpos = dst_gid[mask] - c * cfg.nodes_pad       # sorted pos within core
        order_e = np.argsort(dpos, kind="stable")
        sp = dpos[order_e]
        sv = sg[order_e]
        seg_start = np.searchsorted(sp, np.arange(cfg.nodes_pad))
        rank = np.arange(len(sp)) - seg_start[sp]

        idx_arr = np.zeros((P, slot_cols), dtype=np.int64)
        # init every slot with the node's own gid (safe row)
        own = (c * cfg.nodes_pad + np.arange(cfg.nodes_pad)).reshape(cfg.n_tiles, P)
        for t in range(cfg.n_tiles):
            idx_arr[:, offs[t]:offs[t + 1]] = own[t][:, None]
        col = offs[sp // P] + rank
        idx_arr[sp % P, col] = sv
        # padding slots replicate slot 0 of the node
        ds = deg_sorted[c].reshape(cfg.n_tiles, P)
        npad_arr = np.zeros((P, cfg.n_tiles), dtype=np.float32)
        for t in range(cfg.n_tiles):
            D = int(Dts[t])
            blk = idx_arr[:, offs[t]:offs[t + 1]]
            degs = ds[t]                                # [P]
            pad_mask = np.arange(D)[None, :] >= np.maximum(degs, 1)[:, None]
            first = blk[:, 0:1]
            blk[pad_mask] = np.broadcast_to(first, blk.shape)[pad_mask]
            idx_arr[:, offs[t]:offs[t + 1]] = blk
            npad_arr[:, t] = D - np.maximum(degs, 1)
        idxs_all.append(idx_arr.astype(np.int32))
        npad_all.append(npad_arr)

    return Dts, groups, offs, orders, idxs_all, npad_all


def make_in_maps(inputs, cfg, Dts, offs, orders, idxs_all, npad_all):
    x = np.asarray(inputs["x"], dtype=np.float32)
    W0 = np.asarray(inputs["W0"], dtype=np.float32)
    W_rest = np.asarray(inputs["W_rest"], dtype=np.float32)
    att_src = np.asarray(inputs["att_src"], dtype=np.float32)
    att_dst = np.asarray(inputs["att_dst"], dtype=np.float32)
    bias = np.asarray(inputs["bias"], dtype=np.float32)
    L, h = cfg.L, cfg.h

    w0_pad = np.zeros((cfg.f_in_pad, h), np.float32)
    w0_pad[:cfg.f_in] = W0
    wrest = W_rest.reshape(max(L - 1, 1) * h, h) if L > 1 else np.zeros((h, h), np.float32)
    a_s = att_src.reshape(L, h)
    a_d = att_dst.reshape(L, h)
    asrep = np.repeat(a_s[:, None, :], P, axis=1).reshape(L * P, h)
    adrep = np.repeat(a_d[:, None, :], P, axis=1).reshape(L * P, h)
    brep = np.repeat(bias[:, None, :], P, axis=1).reshape(L * P, h)

    in_maps = []
    for c in range(cfg.n_cores):
        lo = c * cfg.nodes_real
        xc = x[lo:lo + cfg.nodes_real][orders[c]]       # [nodes_real, f_in]
        xT = np.zeros((cfg.f_in_pad, cfg.nodes_pad), np.float32)
        xT[:cfg.f_in, :cfg.nodes_real] = xc.T
        in_maps.append({
            "xT": xT, "idxs": idxs_all[c], "npad": npad_all[c],
            "w0": w0_pad, "wrest": wrest,
            "asrep": asrep, "adrep": adrep, "brep": brep,
        })
    return in_maps


def unshard(results, cfg, orders):
    n_real = cfg.nodes_real * cfg.n_cores
    out = np.empty((n_real, cfg.h), np.float32)
    for c in range(cfg.n_cores):
        oc = results[c]["out"][:cfg.nodes_real]
        out[c * cfg.nodes_real + orders[c]] = oc
    return out


_CACHE = {}


def kernel(**inputs):
    cfg = Cfg()
    edge_index = np.asarray(inputs["edge_index"])
    Dts, groups, offs, orders, idxs_all, npad_all = preprocess(edge_index, cfg)
    key = tuple(Dts.tolist())
    if key not in _CACHE:
        _CACHE[key] = build_nc(cfg, Dts, groups)
    nc = _CACHE[key]
    in_maps = make_in_maps(inputs, cfg, Dts, offs, orders, idxs_all, npad_all)
    res = bass_utils.run_bass_kernel_spmd(nc, in_maps,
                                          core_ids=list(range(cfg.n_cores)))
    return unshard(res.results, cfg, orders)



# revision 23
# speedup vs baseline: 1.7650x; 1.7650x over previous
"""GAT (5-layer, 41 heads, max-aggr) on 8 trn2 NeuronCores — fp16 rewrite.

Strategy (dst-sharded graph parallel, v2):
  - nodes sharded contiguously (12500/core, padded to 12544 = 98*128);
    per-core in-degree sort so 128-node tiles have uniform segment length.
  - fp16 feature table [100352, 42] (col 41 zero-pad for 4B row alignment);
    per layer each core writes its z shard and AllGathers in TWO halves so
    the first AG overlaps edge compute.
  - ELL tiles are grouped (group = gn tiles sharing width Dg, gn*Dg <=
    slot_budget) and slots are d-major interleaved, so one indirect DMA
    fetches a whole group and all edge-stage ops are dense 2D/3D slices:
      e = g*a_s + ad ; lrelu (DVE stt) ; exp (ACT) ; gm = ex*g
      segment-sum / segment-max via in-place pairwise TREE over the d axis
      (contiguous fp16 2x ops instead of strided 1x tensor_reduce).
  - matmul: layer 0 head-major (k-tiled over 640) + PE transpose; layers
    1-4 node-major (lhsT = actT block) so z needs no transpose; relu is
    fused into the actT transpose-copy; 1/denom = exp(-ln(denom)) on ACT.
"""
import sys
for _p in ("/opt/trn_rl_repo",):
    if _p not in sys.path:
        sys.path.insert(0, _p)

import numpy as np
from contextlib import ExitStack

from concourse import bass, mybir, tile, bacc, bass_utils
from concourse.masks import make_identity

F32 = mybir.dt.float32
F16 = mybir.dt.float16
I32 = mybir.dt.int32
ALU = mybir.AluOpType
AF = mybir.ActivationFunctionType
AX = mybir.AxisListType

P = 128
NEG_SLOPE = 0.2


class Cfg:
    def __init__(self, n_cores=8, nodes_real=12500, n_tiles=98, f_in=602,
                 f_in_pad=640, h=41, L=5, mchunk=512, reps=1, ablate="",
                 slot_budget=192, max_group=32, half_rows=6144, tap=""):
        self.tap = tap
        self.n_cores = n_cores
        self.nodes_real = nodes_real
        self.n_tiles = n_tiles
        self.nodes_pad = n_tiles * P          # 12544
        self.f_in = f_in
        self.f_in_pad = f_in_pad
        self.kt = f_in_pad // P               # 5
        self.h = h                            # 41
        self.h2 = h + 1                       # 42 (pad col for alignment)
        self.L = L
        self.mchunk = mchunk
        self.reps = reps
        self.ablate = ablate
        self.slot_budget = slot_budget
        self.max_group = max_group
        self.half_rows = half_rows            # AG split point (node rows)
        self.vg = self.nodes_pad * n_cores    # 100352 table rows


# ----------------------------------------------------------------- builder --
def build_nc(cfg, Dts, groups):
    nt, h, h2, L = cfg.n_tiles, cfg.h, cfg.h2, cfg.L
    HR = cfg.half_rows                        # 6144
    HRB = cfg.nodes_pad - HR                  # 6400
    HTA = HR // P                             # 48 tiles in half A
    slot_cols = int(sum(gn * Dg for (_t0, gn, Dg) in groups))
    s_max = int(max(gn * Dg for (_t0, gn, Dg) in groups))
    gn_max = int(max(gn for (_t0, gn, Dg) in groups))

    nc = bacc.Bacc("TRN2", target_bir_lowering=False, debug=False,
                   num_devices=cfg.n_cores)

    xT = nc.dram_tensor("xT", [cfg.f_in_pad, cfg.nodes_pad], F16, kind="ExternalInput")
    idxs = nc.dram_tensor("idxs", [P, slot_cols], I32, kind="ExternalInput")
    npad1 = nc.dram_tensor("npad1", [P, nt * h2], F16, kind="ExternalInput")
    w0 = nc.dram_tensor("w0", [cfg.f_in_pad, h], F16, kind="ExternalInput")
    wrest = nc.dram_tensor("wrest", [max(L - 1, 1) * h, h], F16, kind="ExternalInput")
    asrep = nc.dram_tensor("asrep", [L * P, h2], F16, kind="ExternalInput")
    adrep = nc.dram_tensor("adrep", [L * P, h2], F16, kind="ExternalInput")
    brep = nc.dram_tensor("brep", [L * P, h2], F16, kind="ExternalInput")
    out_d = nc.dram_tensor("out", [cfg.nodes_pad, h], F16, kind="ExternalOutput")
    dbg = (nc.dram_tensor("dbg", [P, 40960], F16, kind="ExternalOutput")
           if cfg.tap else None)

    groups_rg = [list(range(cfg.n_cores))]
    TA_ROWS = HR * cfg.n_cores                # table rows in half A

    with tile.TileContext(nc) as tc, ExitStack() as ctx:
        const = ctx.enter_context(tc.tile_pool(name="const", bufs=1))
        actp = ctx.enter_context(tc.tile_pool(name="actp", bufs=1))
        zp = ctx.enter_context(tc.tile_pool(name="zp", bufs=1))
        adp = ctx.enter_context(tc.tile_pool(name="adp", bufs=2))
        outp = ctx.enter_context(tc.tile_pool(name="outp", bufs=2))
        rhsp = ctx.enter_context(tc.tile_pool(name="rhsp", bufs=3))
        ztp = ctx.enter_context(tc.tile_pool(name="ztp", bufs=2))
        gp = ctx.enter_context(tc.tile_pool(name="gp", bufs=2))
        ep = ctx.enter_context(tc.tile_pool(name="ep", bufs=1))
        gmp = ctx.enter_context(tc.tile_pool(name="gmp", bufs=1))
        smp = ctx.enter_context(tc.tile_pool(name="smp", bufs=2))
        psmm = ctx.enter_context(tc.tile_pool(name="psmm", bufs=2, space="PSUM"))
        psnd = ctx.enter_context(tc.tile_pool(name="psnd", bufs=2, space="PSUM"))
        pstr = ctx.enter_context(tc.tile_pool(name="pstr", bufs=2, space="PSUM"))
        dram = ctx.enter_context(tc.tile_pool(name="dram", bufs=2, space="DRAM"))

        # --------------------------------------------------------- constants
        ident = const.tile([P, P], F16)
        make_identity(nc, ident[:])
        idx_sb = const.tile([P, slot_cols], I32)
        nc.sync.dma_start(out=idx_sb[:], in_=idxs[:])
        npad1_sb = const.tile([P, nt * h2], F16)
        nc.sync.dma_start(out=npad1_sb[:], in_=npad1[:])
        as_sb = const.tile([P, L * h2], F16)
        nc.sync.dma_start(out=as_sb[:].rearrange("p (l h) -> p l h", l=L),
                          in_=asrep[:].rearrange("(l p) h -> p l h", p=P))
        ad_sb = const.tile([P, L * h2], F16)
        nc.sync.dma_start(out=ad_sb[:].rearrange("p (l h) -> p l h", l=L),
                          in_=adrep[:].rearrange("(l p) h -> p l h", p=P))
        b_sb = const.tile([P, L * h2], F16)
        nc.sync.dma_start(out=b_sb[:].rearrange("p (l h) -> p l h", l=L),
                          in_=brep[:].rearrange("(l p) h -> p l h", p=P))
        w0_sb = const.tile([P, cfg.kt * h], F16)
        nc.sync.dma_start(out=w0_sb[:].rearrange("p (k h) -> p k h", k=cfg.kt),
                          in_=w0[:].rearrange("(k p) h -> p k h", p=P))
        wr_sb = const.tile([h, max(L - 1, 1) * h], F16)
        nc.sync.dma_start(out=wr_sb[:].rearrange("p (l h) -> p l h", l=max(L - 1, 1)),
                          in_=wrest[:].rearrange("(l p) h -> p l h", p=h))

        # z_sb persists (pad col 41 must stay zero) -> single slot, memset once
        z_sb = zp.tile([P, nt * h2], F16, tag="z_sb")
        nc.vector.memset(z_sb[:], 0.0)

        # group slot-column offsets
        goffs = np.concatenate(
            [[0], np.cumsum([gn * Dg for (_t0, gn, Dg) in groups])]).astype(int)

        # m-chunk list
        mlist = []
        m0 = 0
        while m0 < cfg.nodes_pad:
            mw = min(cfg.mchunk, cfg.nodes_pad - m0)
            mlist.append((m0, mw))
            m0 += mw

        def tree_reduce(buf, m, width, op, final_out=None):
            """In-place pairwise tree over d-major blocks of `width` cols.

            buf[:, d*width:(d+1)*width] for d in [0, m). Result in block 0
            (or written to final_out fp32 on the last combine for sum)."""
            while m > 1:
                if m % 2 == 1:
                    nc.vector.tensor_tensor(
                        out=buf[:, 0:width], in0=buf[:, 0:width],
                        in1=buf[:, (m - 1) * width:m * width], op=op)
                    m -= 1
                    if m == 1:
                        break
                half = m // 2
                if half == 1 and final_out is not None:
                    nc.vector.tensor_tensor(
                        out=final_out, in0=buf[:, 0:width],
                        in1=buf[:, width:2 * width], op=op)
                    return True
                nc.vector.tensor_tensor(
                    out=buf[:, 0:half * width],
                    in0=buf[:, 0:half * width],
                    in1=buf[:, half * width:m * width], op=op)
                m = half
            return False

        def producer_chunk(lp, ci, actT, out_prev):
            """Build z for layer lp, m-chunk ci. Returns nothing.

            lp==0: head-major k-tiled matmul from streamed xT + PE transpose.
            lp>=1: node-major matmul from actT (written here from out_prev)."""
            m0, mw = mlist[ci]
            njt = mw // P
            if lp == 0:
                ps0 = psmm.tile([h, cfg.mchunk], F32, tag="mm0")
                for k in range(cfg.kt):
                    rhs = rhsp.tile([P, cfg.mchunk], F16, tag="rhs")
                    nc.sync.dma_start(out=rhs[:, :mw],
                                      in_=xT[k * P:(k + 1) * P, m0:m0 + mw])
                    nc.tensor.matmul(ps0[:, :mw], lhsT=w0_sb[:, k * h:(k + 1) * h],
                                     rhs=rhs[:, :mw], start=(k == 0),
                                     stop=(k == cfg.kt - 1))
                zt = ztp.tile([h, cfg.mchunk], F16, tag="zt")
                nc.scalar.copy(out=zt[:, :mw], in_=ps0[:, :mw])
                psN = psnd.tile([P, 4 * h2], F16, tag="psN0")
                for j in range(njt):
                    nc.tensor.transpose(out=psN[:, j * h2:j * h2 + h],
                                        in_=zt[:, j * P:(j + 1) * P],
                                        identity=ident[:h, :h])
            else:
                # transpose prev-layer out (with relu) into actT, then matmul
                pA = pstr.tile([h, 4 * P], F16, tag="pA")
                for j in range(njt):
                    nc.tensor.transpose(
                        out=pA[:, j * P:(j + 1) * P],
                        in_=out_prev[:, (4 * ci + j) * h2:(4 * ci + j) * h2 + h],
                        identity=ident[:])
                nc.scalar.activation(out=actT[:, m0:m0 + mw], in_=pA[:, :mw],
                                     func=AF.Relu)
                psN = psnd.tile([P, 4 * h], F32, tag="psN")
                for j in range(njt):
                    nc.tensor.matmul(
                        psN[:, j * h:(j + 1) * h],
                        lhsT=actT[:, m0 + j * P:m0 + (j + 1) * P],
                        rhs=wr_sb[:, (lp - 1) * h:lp * h],
                        start=True, stop=True)
            # node-major z chunk -> z_sb (pad col untouched, stays 0)
            if lp == 0:
                src = psN[:, :njt * h2].rearrange(
                    "p (t h) -> p t h", h=h2)[:, :, 0:h]
            else:
                src = psN[:, :njt * h].rearrange("p (t h) -> p t h", h=h)
            nc.scalar.copy(
                out=z_sb[:].rearrange("p (t h) -> p t h", h=h2)[
                    :, 4 * ci:4 * ci + njt, 0:h],
                in_=src)

        def emit_bounce(lp, table):
            bounce = dram.tile([cfg.nodes_pad, h2], F16, tag="bounce")
            nc.sync.dma_start(
                out=bounce[:].rearrange("(t p) c -> p t c", p=P),
                in_=z_sb[:].rearrange("p (t c) -> p t c", c=h2))
            nc.gpsimd.collective_compute(
                "AllGather", ALU.bypass, replica_groups=groups_rg,
                ins=[bounce[:]], outs=[table[:]])

        def emit_ad_all(lp):
            ad_all = adp.tile([P, nt * h2], F16, tag="ad_all")
            a_dl = ad_sb[:, lp * h2:(lp + 1) * h2]
            nc.vector.tensor_tensor(
                out=ad_all[:].rearrange("p (t h) -> p t h", h=h2),
                in0=z_sb[:].rearrange("p (t h) -> p t h", h=h2),
                in1=a_dl.unsqueeze(1).broadcast_to([P, nt, h2]), op=ALU.mult)
            return ad_all

        def tapf(off, ap2d, width):
            if dbg is not None:
                nc.sync.dma_start(out=dbg[:, off:off + width], in_=ap2d)

        def edge_group(l, gi, table, ad_all, out_all, tapped=False):
            t0, gn, Dg = groups[gi]
            S = gn * Dg
            W = gn * h2                        # block width (cols) per d level
            c0 = int(goffs[gi])
            gbuf = gp.tile([P, s_max * h2], F16, tag="g")
            if cfg.ablate == "nogather":
                nc.vector.memset(gbuf[:, :S * h2], 1.0)
            elif gi == 0:
                # split first gather to shorten the post-AG descriptor bubble
                sh = S // 2
                nc.gpsimd.indirect_dma_start(
                    out=gbuf[:, :sh * h2], out_offset=None, in_=table[:],
                    in_offset=bass.IndirectOffsetOnAxis(
                        ap=idx_sb[:, c0:c0 + sh], axis=0))
                nc.gpsimd.indirect_dma_start(
                    out=gbuf[:, sh * h2:S * h2], out_offset=None, in_=table[:],
                    in_offset=bass.IndirectOffsetOnAxis(
                        ap=idx_sb[:, c0 + sh:c0 + S], axis=0))
            else:
                nc.gpsimd.indirect_dma_start(
                    out=gbuf[:, :S * h2], out_offset=None, in_=table[:],
                    in_offset=bass.IndirectOffsetOnAxis(
                        ap=idx_sb[:, c0:c0 + S], axis=0))
            if tapped:
                tapf(8232, gbuf[:, :S * h2], S * h2)
            ebuf = ep.tile([P, s_max * h2], F16, tag="e")
            gmb = gmp.tile([P, s_max * h2], F16, tag="gm")
            a_sl = as_sb[:, l * h2:(l + 1) * h2]
            # e = g * a_s  (dense [p, S, h2])
            nc.vector.tensor_tensor(
                out=ebuf[:, :S * h2].rearrange("p (s h) -> p s h", h=h2),
                in0=gbuf[:, :S * h2].rearrange("p (s h) -> p s h", h=h2),
                in1=a_sl.unsqueeze(1).broadcast_to([P, S, h2]), op=ALU.mult)
            # e += ad (broadcast over d levels)
            nc.vector.tensor_tensor(
                out=ebuf[:, :S * h2].rearrange("p (d x) -> p d x", d=Dg),
                in0=ebuf[:, :S * h2].rearrange("p (d x) -> p d x", d=Dg),
                in1=ad_all[:, t0 * h2:(t0 + gn) * h2].unsqueeze(1)
                    .broadcast_to([P, Dg, W]), op=ALU.add)
            # leaky relu on DVE: e = max(0.2*e, e); clamp to 4 so fp16 exp
            # stays finite on padded/garbage rows (true logits are ~N(0,0.35),
            # so the clamp never binds on real data)
            nc.vector.scalar_tensor_tensor(
                out=ebuf[:, :S * h2], in0=ebuf[:, :S * h2], scalar=NEG_SLOPE,
                in1=ebuf[:, :S * h2], op0=ALU.mult, op1=ALU.max)
            nc.vector.tensor_scalar_min(out=ebuf[:, :S * h2],
                                        in0=ebuf[:, :S * h2], scalar1=4.0)
            # ex = exp(e) on ACT
            nc.scalar.activation(out=ebuf[:, :S * h2], in_=ebuf[:, :S * h2],
                                 func=AF.Exp)
            if tapped:
                tapf(16212, ebuf[:, :S * h2], S * h2)
            # gm = ex * g
            nc.vector.tensor_tensor(out=gmb[:, :S * h2], in0=ebuf[:, :S * h2],
                                    in1=gbuf[:, :S * h2], op=ALU.mult)
            if tapped:
                tapf(24192, gmb[:, :S * h2], S * h2)
            # fold padding correction into slot level 0: ex0 *= (1 - npad)
            nc.vector.tensor_tensor(
                out=ebuf[:, 0:W], in0=ebuf[:, 0:W],
                in1=npad1_sb[:, t0 * h2:(t0 + gn) * h2], op=ALU.mult)
            # segment sum / segment max over d (pairwise trees)
            s_g = smp.tile([P, gn_max * h2], F32, tag="s_g")
            wrote = tree_reduce(ebuf, Dg, W, ALU.add, final_out=s_g[:, :W])
            if not wrote:  # Dg small; cast remaining block to f32
                nc.scalar.copy(out=s_g[:, :W], in_=ebuf[:, 0:W])
            tree_reduce(gmb, Dg, W, ALU.max)
            # tail: out = gm_max * exp(-ln(denom)) + b
            # floor the denom (true minimum ~0.6; garbage rows can drive the
            # pad-corrected sum negative, which would NaN the Ln)
            nc.vector.tensor_scalar_max(out=s_g[:, :W], in0=s_g[:, :W],
                                        scalar1=0.05)
            ls_g = smp.tile([P, gn_max * h2], F16, tag="ls_g")
            nc.scalar.activation(out=ls_g[:, :W], in_=s_g[:, :W], func=AF.Ln)
            rs_g = smp.tile([P, gn_max * h2], F16, tag="rs_g")
            nc.scalar.activation(out=rs_g[:, :W], in_=ls_g[:, :W], func=AF.Exp,
                                 scale=-1.0)
            osl = out_all[:, t0 * h2:(t0 + gn) * h2]
            nc.vector.tensor_tensor(out=osl, in0=gmb[:, 0:W], in1=rs_g[:, :W],
                                    op=ALU.mult)
            b_l = b_sb[:, l * h2:(l + 1) * h2]
            nc.vector.tensor_tensor(
                out=osl.rearrange("p (g h) -> p g h", h=h2),
                in0=osl.rearrange("p (g h) -> p g h", h=h2),
                in1=b_l.unsqueeze(1).broadcast_to([P, gn, h2]), op=ALU.add)
            # clamp activations (true range ~±12) so garbage rows can't
            # overflow fp16 through the layer recursion
            nc.vector.tensor_scalar(out=osl, in0=osl, scalar1=30.0,
                                    scalar2=-30.0, op0=ALU.min, op1=ALU.max)
            if tapped:
                tapf(32172, osl, gn * h2)

        def logsoftmax(out_all):
            w3 = lambda ap: ap.rearrange("p (t h) -> p t h", h=h2)
            x3 = out_all[:].rearrange("p (t h) -> p t h", h=h2)[:, :, 0:h]
            mx = smp.tile([P, nt], F16, tag="mx")
            nc.vector.tensor_reduce(out=mx[:], in_=x3, axis=AX.X, op=ALU.max)
            nc.vector.tensor_tensor(
                out=x3, in0=x3,
                in1=mx[:].unsqueeze(2).broadcast_to([P, nt, h]), op=ALU.subtract)
            exl = gmp.tile([P, s_max * h2], F16, tag="gm")
            e3 = exl[:, :nt * h].rearrange("p (t h) -> p t h", h=h)
            nc.scalar.activation(out=e3, in_=x3, func=AF.Exp)
            sl = smp.tile([P, nt], F32, tag="sl")
            nc.vector.tensor_reduce(out=sl[:], in_=e3, axis=AX.X, op=ALU.add)
            sl16 = smp.tile([P, nt], F16, tag="sl16")
            nc.scalar.activation(out=sl16[:], in_=sl[:], func=AF.Ln)
            nc.vector.tensor_tensor(
                out=x3, in0=x3,
                in1=sl16[:].unsqueeze(2).broadcast_to([P, nt, h]),
                op=ALU.subtract)
            nc.sync.dma_start(
                out=out_d[:].rearrange("(t p) h -> p t h", p=P),
                in_=x3)

        # ------------------------------------------------------------ main --
        for _rep in range(cfg.reps):
            # layer-0 producer
            table = dram.tile([cfg.vg, h2], F16, tag="table", addr_space="Shared")
            for ci in range(len(mlist)):
                producer_chunk(0, ci, None, None)
            if dbg is not None and _rep == 0:
                tapf(0, z_sb[:, :nt * h2], nt * h2)
            emit_bounce(0, table)
            ad_all = emit_ad_all(0)
            if dbg is not None and _rep == 0:
                tapf(4116, ad_all[:, :nt * h2], nt * h2)

            for l in range(L):
                out_all = outp.tile([P, nt * h2], F16, tag="out_all")
                if l < L - 1:
                    actT = actp.tile([h, cfg.nodes_pad], F16, tag="actT")
                    table_n = dram.tile([cfg.vg, h2], F16, tag="table",
                                        addr_space="Shared")
                cover = 0
                for gi in range(len(groups)):
                    if cfg.ablate == "noedge":
                        if gi == 0:
                            nc.scalar.copy(out=out_all[:], in_=z_sb[:])
                    else:
                        edge_group(l, gi, table, ad_all, out_all,
                                   tapped=(dbg is not None and _rep == 0
                                           and l == 0 and gi == 0))
                    t_done = groups[gi][0] + groups[gi][1]
                    if l < L - 1:
                        while cover < len(mlist) and \
                                min(4 * cover + 4, nt) <= t_done:
                            producer_chunk(l + 1, cover, actT, out_all)
                            cover += 1
                if l < L - 1:
                    while cover < len(mlist):
                        producer_chunk(l + 1, cover, actT, out_all)
                        cover += 1
                    emit_bounce(l + 1, table_n)
                    ad_all = emit_ad_all(l + 1)
                    table = table_n
                else:
                    logsoftmax(out_all)

    nc.compile()
    return nc


# ------------------------------------------------------------ preprocessing --
def preprocess(edge_index, cfg):
    """Shard + degree-sort + grouped/interleaved ELL-pack. Per-core arrays."""
    n_real = cfg.nodes_real * cfg.n_cores
    src = np.concatenate([edge_index[0], np.arange(n_real, dtype=np.int64)])
    dst = np.concatenate([edge_index[1], np.arange(n_real, dtype=np.int64)])
    deg = np.bincount(dst, minlength=n_real)

    orders = []
    sorted_pos = np.empty(n_real, dtype=np.int64)   # node -> sorted pos in core
    for c in range(cfg.n_cores):
        lo = c * cfg.nodes_real
        d = deg[lo:lo + cfg.nodes_real]
        order = np.argsort(-d, kind="stable")
        orders.append(order)
        sorted_pos[lo + order] = np.arange(cfg.nodes_real)

    # global table row of each node (rank-concatenated AG layout)
    def gid_of(core, spos):
        return core * cfg.nodes_pad + np.asarray(spos)

    gid_of_node = np.empty(n_real, dtype=np.int64)
    for c in range(cfg.n_cores):
        lo = c * cfg.nodes_real
        gid_of_node[lo:lo + cfg.nodes_real] = gid_of(c, sorted_pos[lo:lo + cfg.nodes_real])

    # unified per-tile ELL widths
    Dts = np.zeros(cfg.n_tiles, dtype=np.int64)
    deg_sorted = []
    for c in range(cfg.n_cores):
        lo = c * cfg.nodes_real
        ds = deg[lo:lo + cfg.nodes_real][orders[c]]
        ds = np.concatenate([ds, np.zeros(cfg.nodes_pad - cfg.nodes_real, np.int64)])
        deg_sorted.append(ds)
        Dts = np.maximum(Dts, ds.reshape(cfg.n_tiles, P).max(1))
    Dts = np.maximum(Dts, 1)

    # grouping: gn tiles share width Dg, gn*Dg <= slot_budget
    groups = []
    t = 0
    while t < cfg.n_tiles:
        Dg = int(Dts[t])
        n = 1
        while (t + n < cfg.n_tiles and n < cfg.max_group
               and (n + 1) * Dg <= cfg.slot_budget):
            n += 1
        groups.append((t, n, Dg))
        Dts[t:t + n] = Dg
        t += n

    offs = np.concatenate([[0], np.cumsum(Dts)]).astype(int)   # per-tile layout
    slot_cols = int(offs[-1])

    owner = dst // cfg.nodes_real
    src_gid = gid_of_node[src]

    idxs_all, npad_all = [], []
    for c in range(cfg.n_cores):
        mask = owner == c
        sg = src_gid[mask]
        dpos = sorted_pos[dst[mask]]                  # sorted pos within core
        order_e = np.argsort(dpos, kind="stable")
        sp = dpos[order_e]
        sv = sg[order_e]
        seg_start = np.searchsorted(sp, np.arange(cfg.nodes_pad))
        rank = np.arange(len(sp)) - seg_start[sp]

        # per-tile layout first
        idx_arr = np.zeros((P, slot_cols), dtype=np.int64)
        own = gid_of(c, np.arange(cfg.nodes_pad)).reshape(cfg.n_tiles, P)
        for t in range(cfg.n_tiles):
            idx_arr[:, offs[t]:offs[t + 1]] = own[t][:, None]
        col = offs[sp // P] + rank
        idx_arr[sp % P, col] = sv
        ds = deg_sorted[c].reshape(cfg.n_tiles, P)
        npad_arr = np.zeros((P, cfg.n_tiles * cfg.h2), dtype=np.float16)
        for t in range(cfg.n_tiles):
            D = int(Dts[t])
            blk = idx_arr[:, offs[t]:offs[t + 1]]
            degs = ds[t]
            pad_mask = np.arange(D)[None, :] >= np.maximum(degs, 1)[:, None]
            first = blk[:, 0:1]
            blk[pad_mask] = np.broadcast_to(first, blk.shape)[pad_mask]
            idx_arr[:, offs[t]:offs[t + 1]] = blk
            npad_val = 1.0 - (D - np.maximum(degs, 1)).astype(np.float32)
            npad_arr[:, t * cfg.h2:t * cfg.h2 + cfg.h] = \
                npad_val[:, None].astype(np.float16)
        # interleave columns d-major within each group
        idx_il = np.empty_like(idx_arr)
        for (t0, gn, Dg) in groups:
            a = offs[t0]
            blk = idx_arr[:, a:a + gn * Dg].reshape(P, gn, Dg)
            idx_il[:, a:a + gn * Dg] = blk.transpose(0, 2, 1).reshape(P, gn * Dg)
        # keep block reads in-table under the scalar-offset DGE lowering
        np.minimum(idx_il, cfg.vg - 256, out=idx_il)
        idxs_all.append(idx_il.astype(np.int32))
        npad_all.append(npad_arr)

    return Dts, groups, offs, orders, idxs_all, npad_all


def make_in_maps(inputs, cfg, Dts, offs, orders, idxs_all, npad_all):
    x = np.asarray(inputs["x"], dtype=np.float32)
    W0 = np.asarray(inputs["W0"], dtype=np.float32)
    W_rest = np.asarray(inputs["W_rest"], dtype=np.float32)
    att_src = np.asarray(inputs["att_src"], dtype=np.float32)
    att_dst = np.asarray(inputs["att_dst"], dtype=np.float32)
    bias = np.asarray(inputs["bias"], dtype=np.float32)
    L, h, h2 = cfg.L, cfg.h, cfg.h2

    w0_pad = np.zeros((cfg.f_in_pad, h), np.float16)
    w0_pad[:cfg.f_in] = W0.astype(np.float16)
    wrest = (W_rest.reshape(max(L - 1, 1) * h, h) if L > 1
             else np.zeros((h, h), np.float32)).astype(np.float16)
    a_s = att_src.reshape(L, h)
    a_d = att_dst.reshape(L, h)

    def rep42(a):   # [L, h] -> [L*P, h2] fp16 with zero pad col
        out = np.zeros((L, P, h2), np.float16)
        out[:, :, :h] = a[:, None, :].astype(np.float16)
        return out.reshape(L * P, h2)

    asrep = rep42(a_s)
    adrep = rep42(a_d)
    brep = rep42(bias)

    in_maps = []
    for c in range(cfg.n_cores):
        lo = c * cfg.nodes_real
        xc = x[lo:lo + cfg.nodes_real][orders[c]]
        xT = np.zeros((cfg.f_in_pad, cfg.nodes_pad), np.float16)
        xT[:cfg.f_in, :cfg.nodes_real] = xc.T.astype(np.float16)
        in_maps.append({
            "xT": xT, "idxs": idxs_all[c], "npad1": npad_all[c],
            "w0": w0_pad, "wrest": wrest,
            "asrep": asrep, "adrep": adrep, "brep": brep,
        })
    return in_maps


def unshard(results, cfg, orders):
    n_real = cfg.nodes_real * cfg.n_cores
    out = np.empty((n_real, cfg.h), np.float32)
    for c in range(cfg.n_cores):
        oc = results[c]["out"][:cfg.nodes_real].astype(np.float32)
        out[c * cfg.nodes_real + orders[c]] = oc
    return out


_CACHE = {}


def kernel(**inputs):
    cfg = Cfg()
    edge_index = np.asarray(inputs["edge_index"])
    Dts, groups, offs, orders, idxs_all, npad_all = preprocess(edge_index, cfg)
    key = tuple(Dts.tolist())
    if key not in _CACHE:
        _CACHE[key] = build_nc(cfg, Dts, groups)
    nc = _CACHE[key]
    in_maps = make_in_maps(inputs, cfg, Dts, offs, orders, idxs_all, npad_all)
    res = bass_utils.run_bass_kernel_spmd(nc, in_maps,
                                          core_ids=list(range(cfg.n_cores)))
    return unshard(res.results, cfg, orders)
